# revision 1
# baseline (speedup 1.0000x reference)
"""Trainium2 Bass kernel: 3D affine spatial transformer (affine_grid +
trilinear grid_sample, align_corners=True, zeros padding).

Data parallel: one sample per NeuronCore (8 cores). Per core, output is
processed in 10-wide w-chunk "tasks". One indirect DMA per task-group
(40 tasks x 3 z-slots = 120 partition streams) fetches, per (task, jz),
a contiguous 1180-element stream from a zero-padded channel-interleaved
copy (PV) of src in DRAM, covering the task's (y-band x x-window x 2ch)
window. Exact trilinear weights are hat functions relu(1-|t|) evaluated
densely over the (jz, jy, s) window; a PE matmul with a block-selection
matrix collapses the jz slots and emits per-task outputs.
"""

import numpy as np

import concourse.bass as bass
import concourse.bacc as bacc
import concourse.mybir as mybir
from concourse import tile
from concourse.bass import AP, DynSlice
from concourse.bass_utils import run_bass_kernel_spmd

F32 = mybir.dt.float32
I32 = mybir.dt.int32
AO = mybir.AluOpType

N, C, D, H, W = 8, 2, 96, 160, 160
SRCEL = D * H * W
OUTEL = C * SRCEL

L = 10
KZ, KY, KX = 3, 4, 5
MZ, MY, MX = 4, 4, 16
ZP, YP, XP = D + 2 * MZ, H + 3 * MY, W + 2 * MX      # 104, 172, 192
ZSTR = YP * XP * C                                    # 66048
YSTR = XP * C                                         # 384
PVN = ZP * ZSTR                                       # 6868992
STREAM = (KY - 1) * YSTR + ((L - 1) + KX) * C         # 1180
TPG = 40
NTASK = D * H * (W // L)                              # 245760
NG = NTASK // TPG                                     # 6144
NB = 4
NBLK = NG // NB                                       # 1536
ZB_MAX, YB_MAX, XB_MAX = float(ZP - KZ), 167.0, 178.0

_CACHE = {}


def _build_program():
    P = 128
    nc = bacc.Bacc(None, target_bir_lowering=False)
    src = nc.declare_dram_parameter("src", [C, SRCEL], F32, isOutput=False)
    theta = nc.declare_dram_parameter("theta", [1, 12], F32, isOutput=False)
    out = nc.declare_dram_parameter("out", [1, OUTEL], F32, isOutput=True)
    pv = nc.dram_tensor("pv", [PVN, 1], F32)

    def fb(apx, pairs, extra_off=0):
        """clone AP keeping partition pair, replacing free pairs"""
        return AP(apx.tensor, apx.offset + extra_off,
                  [list(apx.ap[0])] + [list(p) for p in pairs])

    with tile.TileContext(nc) as tc:
        with (
            tc.tile_pool(name="big", bufs=1) as big,
            tc.tile_pool(name="pp", bufs=2, space="PSUM") as pp,
        ):
            # ---------- phase 0: zero-fill PV ----------
            pre_ctx = tc.tile_pool(name="pre", bufs=2)
            pre = pre_ctx.__enter__()
            zt = pre.tile([P, 4096], F32)
            nc.vector.memset(zt[:], 0.0)
            CH = P * 4096
            nfull = PVN // CH
            for i in range(nfull):
                nc.sync.dma_start(out=AP(pv[:].tensor, i * CH, [[1, CH]]),
                                  in_=zt[:])
            rem = PVN - nfull * CH          # 53248 = 128*416
            nc.sync.dma_start(out=AP(pv[:].tensor, nfull * CH, [[1, rem]]),
                              in_=zt[:, :rem // P])

            # ---------- phase 1: build PV (channel interleave) ----------
            for z in range(D):
                for yc in range(2):
                    s0 = pre.tile([80, W], F32, tag="s0")
                    s1 = pre.tile([80, W], F32, tag="s1")
                    off = z * H * W + yc * 80 * W
                    nc.sync.dma_start(out=s0[:], in_=AP(
                        src[:].tensor, off, [[W, 80], [1, W]]))
                    nc.sync.dma_start(out=s1[:], in_=AP(
                        src[:].tensor, SRCEL + off, [[W, 80], [1, W]]))
                    il = pre.tile([80, W * C], F32, tag="il")
                    nc.vector.tensor_copy(out=fb(il[:], [[2, W]]), in_=s0[:])
                    nc.vector.tensor_copy(out=fb(il[:], [[2, W]], 1), in_=s1[:])
                    dst_off = ((z + MZ) * ZSTR + (MY + yc * 80) * YSTR
                               + MX * C)
                    nc.sync.dma_start(
                        out=AP(pv[:].tensor, dst_off, [[YSTR, 80], [1, W * C]]),
                        in_=il[:])

            # ---------- phase 2: scalars & constants ----------
            th0 = big.tile([P, 12], F32)
            nc.sync.dma_start(out=th0[:1, :], in_=theta[:])
            ones1 = big.tile([1, P], F32)
            nc.vector.memset(ones1[:], 1.0)
            thps = pp.tile([P, 12], F32, tag="thps")
            nc.tensor.matmul(out=thps[:], lhsT=ones1[:], rhs=th0[:1, :],
                             start=True, stop=True)
            thb = big.tile([P, 12], F32)
            nc.vector.tensor_copy(out=thb[:], in_=thps[:])

            def thc(j):
                return thb[:, j:j + 1]

            sc = big.tile([P, 20], F32)
            (AX, BX, CXc, OX, AY, BY, CYc, OY, AZ, BZ, CZc, OZ,
             AXM1, SP_X, SP_Y, SP_Z) = range(16)

            def scc(j):
                return sc[:, j:j + 1]

            r = 159.0 / 95.0

            def row(dsti, srci, mulc, a_i, b_i, c_i, osc, oadd):
                # A,B,C,O for one axis. srci = first theta idx of the row
                nc.vector.tensor_copy(out=scc(dsti[0]), in_=thc(srci))
                nc.vector.tensor_copy(out=scc(dsti[1]), in_=thc(srci + 1))
                nc.vector.tensor_scalar_mul(out=scc(dsti[2]),
                                            in0=thc(srci + 2), scalar1=mulc)
                tmp = pre.tile([P, 1], F32, tag="sctmp")
                nc.vector.tensor_tensor(out=tmp[:], in0=thc(srci + 3),
                                        in1=thc(srci), op=AO.subtract)
                nc.vector.tensor_tensor(out=tmp[:], in0=tmp[:],
                                        in1=thc(srci + 1), op=AO.subtract)
                nc.vector.tensor_tensor(out=tmp[:], in0=tmp[:],
                                        in1=thc(srci + 2), op=AO.subtract)
                nc.vector.tensor_scalar(out=scc(dsti[3]), in0=tmp[:],
                                        scalar1=osc, scalar2=osc + oadd,
                                        op0=AO.mult, op1=AO.add)

            # x: A=t00 B=t01 C=t02*r O=79.5*(1+t03-t00-t01-t02)+MX
            row((AX, BX, CXc, OX), 0, r, None, None, None, 79.5, float(MX))
            row((AY, BY, CYc, OY), 4, r, None, None, None, 79.5, float(MY))
            # z: A=t20/r B=t21/r C=t22 O=47.5*(...)+MZ
            nc.vector.tensor_scalar_mul(out=scc(AZ), in0=thc(8), scalar1=1.0 / r)
            nc.vector.tensor_scalar_mul(out=scc(BZ), in0=thc(9), scalar1=1.0 / r)
            nc.vector.tensor_copy(out=scc(CZc), in_=thc(10))
            tmp = pre.tile([P, 1], F32, tag="sctmp2")
            nc.vector.tensor_tensor(out=tmp[:], in0=thc(11), in1=thc(8),
                                    op=AO.subtract)
            nc.vector.tensor_tensor(out=tmp[:], in0=tmp[:], in1=thc(9),
                                    op=AO.subtract)
            nc.vector.tensor_tensor(out=tmp[:], in0=tmp[:], in1=thc(10),
                                    op=AO.subtract)
            nc.vector.tensor_scalar(out=scc(OZ), in0=tmp[:], scalar1=47.5,
                                    scalar2=47.5 + MZ, op0=AO.mult, op1=AO.add)
            nc.vector.tensor_scalar_add(out=scc(AXM1), in0=scc(AX),
                                        scalar1=-1.0)
            nc.vector.tensor_scalar_mul(out=scc(SP_X), in0=scc(AXM1),
                                        scalar1=float(L - 1))
            nc.vector.tensor_scalar_mul(out=scc(SP_Y), in0=scc(AY),
                                        scalar1=float(L - 1))
            nc.vector.tensor_scalar_mul(out=scc(SP_Z), in0=scc(AZ),
                                        scalar1=float(L - 1))

            # per-partition: p, tl=floor(p/3), jz=p-3tl
            pidi = big.tile([P, 1], I32)
            nc.gpsimd.iota(pidi[:], [[0, 1]], base=0, channel_multiplier=1)
            pid = big.tile([P, 1], F32)
            nc.vector.tensor_copy(out=pid[:], in_=pidi[:])
            tl = big.tile([P, 1], F32)
            tli = big.tile([P, 1], I32)
            nc.vector.tensor_scalar(out=tl[:], in0=pid[:], scalar1=-1.0,
                                    scalar2=1.0 / 3.0, op0=AO.add, op1=AO.mult)
            nc.vector.tensor_copy(out=tli[:], in_=tl[:])
            nc.vector.tensor_copy(out=tl[:], in_=tli[:])
            jz = big.tile([P, 1], F32)
            nc.vector.scalar_tensor_tensor(out=jz[:], in0=tl[:], scalar=-3.0,
                                           in1=pid[:], op0=AO.mult, op1=AO.add)
            jzoff = big.tile([P, 1], F32)
            nc.vector.tensor_scalar_mul(out=jzoff[:], in0=jz[:],
                                        scalar1=float(ZSTR))

            # Sel [128, TPG] = (tl == m), zero for idle partitions (tl>=40)
            mio = big.tile([P, TPG], I32)
            nc.gpsimd.iota(mio[:], [[1, TPG]], base=0, channel_multiplier=0)
            miof = big.tile([P, TPG], F32)
            nc.vector.tensor_copy(out=miof[:], in_=mio[:])
            sel = big.tile([P, TPG], F32)
            nc.vector.tensor_tensor(out=sel[:], in0=fb(tl[:], [[0, TPG]]),
                                    in1=miof[:], op=AO.is_equal)

            # iotas
            def iotaf(n, tag):
                ti_ = big.tile([P, n], I32, tag=tag + "i")
                nc.gpsimd.iota(ti_[:], [[1, n]], base=0, channel_multiplier=0)
                tf_ = big.tile([P, n], F32, tag=tag + "f")
                nc.vector.tensor_copy(out=tf_[:], in_=ti_[:])
                return tf_

            wlf = iotaf(L, "wl")
            jyf = iotaf(KY, "jy")
            sxf = iotaf(KX, "sx")

            # global field templates (computed once):
            # zwlG[wl] = Az*wl - jz ; ywlG[jy,wl] = Ay*wl - jy ;
            # xwlG[s,wl] = (Ax-1)*wl - s
            zwlG = big.tile([P, L], F32)
            nc.vector.scalar_tensor_tensor(
                out=zwlG[:], in0=wlf[:], scalar=scc(AZ),
                in1=fb(jz[:], [[0, L]]), op0=AO.mult, op1=AO.subtract)
            ywlG = big.tile([P, KY, L], F32)
            nc.vector.scalar_tensor_tensor(
                out=ywlG[:], in0=fb(wlf[:], [[0, KY], [1, L]]),
                scalar=scc(AY), in1=fb(jyf[:], [[1, KY], [0, L]]),
                op0=AO.mult, op1=AO.subtract)
            xwlG = big.tile([P, KX, L], F32)
            nc.vector.scalar_tensor_tensor(
                out=xwlG[:], in0=fb(wlf[:], [[0, KX], [1, L]]),
                scalar=scc(AXM1), in1=fb(sxf[:], [[1, KX], [0, L]]),
                op0=AO.mult, op1=AO.subtract)

            # ---------- phase 3: per-task residuals + stream indices ----
            idxT = big.tile([P, NG], I32)
            rzT = big.tile([P, NG], F32)
            ryT = big.tile([P, NG], F32)
            rxT = big.tile([P, NG], F32)

            CHG = 512
            for c0 in range(0, NG, CHG):
                n = CHG
                gi = pre.tile([P, n], I32, tag="gi")
                nc.gpsimd.iota(gi[:], [[TPG, n]], base=c0 * TPG,
                               channel_multiplier=0)
                tt = pre.tile([P, n], F32, tag="tt")
                nc.vector.tensor_copy(out=tt[:], in_=gi[:])
                nc.vector.tensor_scalar_add(out=tt[:], in0=tt[:],
                                            scalar1=tl[:])
                ti = pre.tile([P, n], I32, tag="ti")

                def fdiv(outt, int_, dv):
                    nc.vector.tensor_scalar(out=outt, in0=int_,
                                            scalar1=-(dv - 1.0) / 2.0,
                                            scalar2=1.0 / dv, op0=AO.add,
                                            op1=AO.mult)
                    nc.vector.tensor_copy(out=ti[:], in_=outt)
                    nc.vector.tensor_copy(out=outt, in_=ti[:])

                dd = pre.tile([P, n], F32, tag="dd")
                fdiv(dd[:], tt[:], 2560.0)
                rem_ = pre.tile([P, n], F32, tag="rem")
                nc.vector.scalar_tensor_tensor(out=rem_[:], in0=dd[:],
                                               scalar=-2560.0, in1=tt[:],
                                               op0=AO.mult, op1=AO.add)
                hh = pre.tile([P, n], F32, tag="hh")
                fdiv(hh[:], rem_[:], 16.0)
                w0 = pre.tile([P, n], F32, tag="w0")
                nc.vector.scalar_tensor_tensor(out=w0[:], in0=hh[:],
                                               scalar=-16.0, in1=rem_[:],
                                               op0=AO.mult, op1=AO.add)
                nc.vector.tensor_scalar_mul(out=w0[:], in0=w0[:],
                                            scalar1=float(L))

                acci = pre.tile([P, n], F32, tag="acci")

                def base_resid(ai, bi, ci, oi, spi, bmax, resT, strd, first):
                    cin = pre.tile([P, n], F32, tag="cin")
                    nc.vector.tensor_scalar_mul(out=cin[:], in0=w0[:],
                                                scalar1=scc(ai))
                    nc.vector.scalar_tensor_tensor(
                        out=cin[:], in0=hh[:], scalar=scc(bi), in1=cin[:],
                        op0=AO.mult, op1=AO.add)
                    nc.vector.scalar_tensor_tensor(
                        out=cin[:], in0=dd[:], scalar=scc(ci), in1=cin[:],
                        op0=AO.mult, op1=AO.add)
                    nc.vector.tensor_scalar_add(out=cin[:], in0=cin[:],
                                                scalar1=scc(oi))
                    c9 = pre.tile([P, n], F32, tag="c9")
                    nc.vector.tensor_scalar_add(out=c9[:], in0=cin[:],
                                                scalar1=scc(spi))
                    nc.vector.tensor_tensor(out=c9[:], in0=c9[:], in1=cin[:],
                                            op=AO.min)
                    cb = pre.tile([P, n], F32, tag="cb")
                    nc.vector.tensor_scalar_add(out=cb[:], in0=c9[:],
                                                scalar1=-0.499999)
                    nc.vector.tensor_copy(out=ti[:], in_=cb[:])
                    nc.vector.tensor_copy(out=cb[:], in_=ti[:])
                    nc.vector.tensor_scalar_max(out=cb[:], in0=cb[:],
                                                scalar1=0.0)
                    nc.vector.tensor_scalar_min(out=cb[:], in0=cb[:],
                                                scalar1=bmax)
                    nc.vector.tensor_tensor(out=resT[:, c0:c0 + n],
                                            in0=cin[:], in1=cb[:],
                                            op=AO.subtract)
                    if first:
                        nc.vector.tensor_scalar_mul(out=acci[:], in0=cb[:],
                                                    scalar1=float(strd))
                    else:
                        nc.vector.scalar_tensor_tensor(
                            out=acci[:], in0=cb[:], scalar=float(strd),
                            in1=acci[:], op0=AO.mult, op1=AO.add)

                base_resid(AZ, BZ, CZc, OZ, SP_Z, ZB_MAX, rzT, ZSTR, True)
                base_resid(AY, BY, CYc, OY, SP_Y, YB_MAX, ryT, YSTR, False)
                base_resid(AX, BX, CXc, OX, SP_X, XB_MAX, rxT, C, False)
                nc.vector.tensor_scalar_add(out=acci[:], in0=acci[:],
                                            scalar1=jzoff[:])
                nc.vector.tensor_copy(out=idxT[:, c0:c0 + n], in_=acci[:])

            # ---------- phase 4: main loop ----------
            pre_ctx.__exit__(None, None, None)
            gtp_ctx = tc.tile_pool(name="gtp", bufs=1)
            gtp = gtp_ctx.__enter__()
            work_ctx = tc.tile_pool(name="work", bufs=2)
            work = work_ctx.__enter__()
            UNR = 4
            with tc.For_i(0, NBLK // UNR, 1, staggered_reset=True) as ib:
              gts = []
              for u_ in range(UNR):
                  idxfix = gtp.tile([P, NB], I32, tag=f"idxfix{u_}")
                  nc.vector.tensor_copy(
                      out=idxfix[:],
                      in_=idxT[:, DynSlice((ib * UNR + u_) * NB, NB)])
                  gt = gtp.tile([P, NB * STREAM], F32, tag=f"gt{u_}")
                  for j in range(NB):
                      nc.gpsimd.indirect_dma_start(
                          out=gt[:, j * STREAM:(j + 1) * STREAM],
                          out_offset=None,
                          in_=pv[:],
                          in_offset=bass.IndirectOffsetOnAxis(
                              ap=idxfix[:, j:j + 1], axis=0))
                  gts.append(gt)
              for u_ in range(UNR):
                  gt = gts[u_]
                  rzs = rzT[:, DynSlice((ib * UNR + u_) * NB, NB)]
                  rys = ryT[:, DynSlice((ib * UNR + u_) * NB, NB)]
                  rxs = rxT[:, DynSlice((ib * UNR + u_) * NB, NB)]

                  zf = work.tile([P, NB, L], F32, tag="zf")
                  nc.vector.tensor_tensor(
                      out=zf[:], in0=fb(zwlG[:], [[0, NB], [1, L]]),
                      in1=fb(rzs, [[1, NB], [0, L]]), op=AO.add)
                  hz = work.tile([P, NB, L], F32, tag="hz")
                  nc.scalar.activation(hz[:], zf[:],
                                       mybir.ActivationFunctionType.Abs)
                  nc.scalar.activation(hz[:], hz[:],
                                       mybir.ActivationFunctionType.Relu,
                                       bias=1.0, scale=-1.0)
                  yf = work.tile([P, NB, KY * L], F32, tag="yf")
                  nc.vector.tensor_tensor(
                      out=yf[:], in0=fb(ywlG[:], [[0, NB], [1, KY * L]]),
                      in1=fb(rys, [[1, NB], [0, KY * L]]), op=AO.add)
                  hy = work.tile([P, NB, KY * L], F32, tag="hy")
                  nc.scalar.activation(hy[:], yf[:],
                                       mybir.ActivationFunctionType.Abs)
                  nc.scalar.activation(hy[:], hy[:],
                                       mybir.ActivationFunctionType.Relu,
                                       bias=1.0, scale=-1.0)
                  xf = work.tile([P, NB, KX * L], F32, tag="xf")
                  nc.vector.tensor_tensor(
                      out=xf[:], in0=fb(xwlG[:], [[0, NB], [1, KX * L]]),
                      in1=fb(rxs, [[1, NB], [0, KX * L]]), op=AO.add)
                  gx = work.tile([P, NB, KX * L], F32, tag="gx")
                  nc.scalar.activation(gx[:], xf[:],
                                       mybir.ActivationFunctionType.Abs)
                  nc.scalar.activation(gx[:], gx[:],
                                       mybir.ActivationFunctionType.Relu,
                                       bias=1.0, scale=-1.0)
                  nc.vector.tensor_tensor(
                      out=gx[:], in0=gx[:],
                      in1=fb(hz[:], [[L, NB], [0, KX], [1, L]]), op=AO.mult)
                  w3 = work.tile([P, NB, KY, KX, L], F32, tag="w3")
                  for jyv in range(KY):
                      w3s = AP(w3[:].tensor, w3[:].offset + jyv * KX * L,
                               [list(w3[:].ap[0]), [KY * KX * L, NB],
                                [L, KX], [1, L]])
                      nc.vector.tensor_tensor(
                          out=w3s,
                          in0=fb(gx[:], [[KX * L, NB], [L, KX], [1, L]]),
                          in1=AP(hy[:].tensor, hy[:].offset + jyv * L,
                                 [list(hy[:].ap[0]), [KY * L, NB], [0, KX],
                                  [1, L]]),
                          op=AO.mult)

                  rt = work.tile([P, C, NB, L], F32, tag="rt")
                  for ch in range(C):
                      prod = work.tile([P, NB, L, KY, KX], F32, tag="pr")
                      for jyv in range(KY):
                          ps_ = AP(prod[:].tensor, prod[:].offset + jyv * KX,
                                   [list(prod[:].ap[0]), [L * KY * KX, NB],
                                    [KY * KX, L], [1, KX]])
                          dap = AP(gt[:].tensor,
                                   gt[:].offset + ch + jyv * YSTR,
                                   [list(gt[:].ap[0]), [STREAM, NB], [C, L],
                                    [C, KX]])
                          wap = AP(w3[:].tensor, w3[:].offset + jyv * KX * L,
                                   [list(w3[:].ap[0]), [KY * KX * L, NB],
                                    [1, L], [L, KX]])
                          nc.vector.tensor_tensor(out=ps_, in0=dap, in1=wap,
                                                  op=AO.mult)
                      nc.vector.tensor_reduce(
                          out=rt[:, ch, :, :].rearrange("p a b -> p (a b)"),
                          in_=prod[:].rearrange("p g w a b -> p (g w) (a b)"),
                          op=AO.add, axis=mybir.AxisListType.X)

                  ps = pp.tile([TPG, C * NB * L], F32, tag="ps")
                  nc.tensor.matmul(
                      out=ps[:], lhsT=sel[:],
                      rhs=rt[:].rearrange("p c g w -> p (c g w)"),
                      start=True, stop=True)
                  stg = work.tile([TPG, C, NB, L], F32, tag="stg")
                  nc.vector.tensor_copy(
                      out=stg[:].rearrange("p c g w -> p (c g w)"), in_=ps[:])
                  for ch in range(C):
                      dsl = out[0, DynSlice((ib * UNR + u_) * (NB * TPG * L)
                                            + ch * SRCEL, NB * TPG * L)]
                      dst = AP(dsl.tensor, dsl.offset,
                               [[L, TPG], [TPG * L, NB], [1, L]])
                      sap = stg[:]
                      srcap = AP(sap.tensor, sap.offset + ch * NB * L,
                                 [list(sap.ap[0]), [L, NB], [1, L]])
                      nc.sync.dma_start(out=dst, in_=srcap)
            work_ctx.__exit__(None, None, None)
            gtp_ctx.__exit__(None, None, None)

    nc.compile()
    return nc


def kernel(src, theta):
    if "prog" not in _CACHE:
        _CACHE["prog"] = _build_program()
    nc = _CACHE["prog"]
    in_maps = []
    for i in range(N):
        in_maps.append({
            "src": np.ascontiguousarray(src[i].reshape(C, SRCEL),
                                        dtype=np.float32),
            "theta": np.ascontiguousarray(theta[i].reshape(1, 12),
                                          dtype=np.float32),
        })
    res = run_bass_kernel_spmd(nc, in_maps, core_ids=list(range(N)))
    o = np.empty((N, C, D, H, W), dtype=np.float32)
    for i in range(N):
        o[i] = res.results[i]["out"].reshape(C, D, H, W)
    return o



# revision 21
# speedup vs baseline: 1.1816x; 1.1816x over previous
"""Trainium2 Bass kernel: 3D affine spatial transformer (affine_grid +
trilinear grid_sample, align_corners=True, zeros padding).

Data parallel: one sample per NeuronCore (8 cores). Per core, output is
processed per z-slab (96 slabs); a slab's 5120 output pixels form 128
lanes x 40 tasks of 5 consecutive x-pixels. A jy-phase-duplicated,
zero-padded fp16 copy of src in DRAM (PV[zq, y0, x0, c, jy]) lets one
indirect-DMA descriptor per (task, jz) fetch the task's full
(3z x 3y x 7x x 2c) interpolation window as a 42-element contiguous
stream. Exact trilinear weights are dense hat functions relu(1-|t|)
evaluated per task from three per-axis residuals; products reduce on
DVE in fp16 and results DMA out with an fp16->f32 cast.
"""

import numpy as np

import concourse.bass as bass
import concourse.bacc as bacc
import concourse.mybir as mybir
from concourse import tile
from concourse.bass import AP
from concourse.bass_utils import run_bass_kernel_spmd

F32 = mybir.dt.float32
F16 = mybir.dt.float16
I32 = mybir.dt.int32
AO = mybir.AluOpType
AF = mybir.ActivationFunctionType

N, C, D, H, W = 8, 2, 96, 160, 160
HW = H * W
SRCEL = D * H * W
OUTEL = C * SRCEL

LX, KX, KY, KZ = 5, 3, 3, 3
U = LX - 1 + KX                      # 7
INNER = C * KY                       # 6
DSTREAM = U * INNER                  # 42 els per (task, jz) descriptor
PADZ, PADY, PADX = 3, 4, 7
ZPN, YPN, XN = 101, 167, 176
JZSTR = YPN * XN                     # 29392 rows per z plane
ROWS = ZPN * JZSTR                   # 2968592
ROWS_PAD = 2970240
PVELS = ROWS_PAD * INNER             # 17821440 = 34 * (128*4095)
ZLIM, YLIM, XLIM = 98.0, 164.0, 169.0

TY, XIN = 20, 2
NT = TY * XIN                        # 40 tasks per lane per slab
YG, XCP = 8, 16                      # lane = yg*16 + xcp
SLABS = D
CH_SL = 12                           # slabs per phase-3 chunk
NCHUNK = SLABS // CH_SL
TPC = CH_SL * NT                     # 960 tasks/lane/chunk
NTL = SLABS * NT                     # 3840 tasks per lane

_CACHE = {}
DEBUG = False


def fb(apx, pairs, extra_off=0):
    """clone AP keeping partition pair, replacing free pairs"""
    return AP(apx.tensor, apx.offset + extra_off,
              [list(apx.ap[0])] + [list(p) for p in pairs])


def _build_program():
    P = 128
    nc = bacc.Bacc(None, target_bir_lowering=False)
    src = nc.declare_dram_parameter("src", [C, SRCEL], F32, isOutput=False)
    theta = nc.declare_dram_parameter("theta", [1, 12], F32, isOutput=False)
    out = nc.declare_dram_parameter("out", [1, OUTEL], F32, isOutput=True)
    pv = nc.dram_tensor("pv", [ROWS_PAD, INNER], F16)
    if DEBUG:
        dbg_idx = nc.declare_dram_parameter("dbg_idx", [128, NTL], F32,
                                            isOutput=True)
        dbg_r = nc.declare_dram_parameter("dbg_r", [128, NTL * 3], F16,
                                          isOutput=True)
        dbg_dt = nc.declare_dram_parameter("dbg_dt", [128, NT * KZ * DSTREAM],
                                           F16, isOutput=True)
        dbg_hat = nc.declare_dram_parameter("dbg_hat", [128, NT * 45], F16,
                                            isOutput=True)
        dbg_prod = nc.declare_dram_parameter("dbg_prod",
                                             [128, NT * C * LX * 27], F16,
                                             isOutput=True)
        dbg_red2 = nc.declare_dram_parameter("dbg_red2", [128, C * NT * LX],
                                             F16, isOutput=True)
        dbg_misc = nc.declare_dram_parameter("dbg_misc", [128, 224], F32,
                                             isOutput=True)
        dbg_pv = nc.declare_dram_parameter("dbg_pv", [128, 4096], F16,
                                           isOutput=True)

    r = 79.5 / 47.5

    with tile.TileContext(nc) as tc:
        with (
            tc.tile_pool(name="per", bufs=1) as per,
            tc.tile_pool(name="pp", bufs=1, space="PSUM") as pp,
        ):
            pre_ctx = tc.tile_pool(name="pre", bufs=2)
            pre = pre_ctx.__enter__()

            # ---------- P0: zero-fill PV ----------
            zt = pre.tile([P, 4095], F16)
            nc.vector.memset(zt[:], 0.0)
            CH = P * 4095
            for i in range(PVELS // CH):
                nc.sync.dma_start(out=AP(pv[:].tensor, i * CH, [[1, CH]]),
                                  in_=zt[:])

            # ---------- P2: scalars, templates ----------
            th0 = per.tile([P, 12], F32)
            nc.sync.dma_start(out=th0[:1, :], in_=theta[:])
            ones1 = per.tile([1, P], F32)
            nc.vector.memset(ones1[:], 1.0)
            thps = pp.tile([P, 12], F32, tag="thps")
            nc.tensor.matmul(out=thps[:], lhsT=ones1[:], rhs=th0[:1, :],
                             start=True, stop=True)
            thb = per.tile([P, 12], F32)
            nc.vector.tensor_copy(out=thb[:], in_=thps[:])

            def thc(j):
                return thb[:, j:j + 1]

            # per-axis scalars A,B,C,O' (O' includes +pad)
            sc = per.tile([P, 24], F32)
            (AZ, BZ, CZ, OZ, AY, BY, CY, OY, AX, BX, CX, OX, AXM1,
             MZB, MYB, MXB) = range(16)

            def scc(j):
                return sc[:, j:j + 1]

            tmp1 = per.tile([P, 1], F32, tag="tmp1")

            def mkrow(dst, srci, cmul, osc, oadd):
                # A,B,C,O for one axis; A=t[srci]*amul handled by caller
                nc.vector.tensor_copy(out=scc(dst[0]), in_=thc(srci))
                nc.vector.tensor_copy(out=scc(dst[1]), in_=thc(srci + 1))
                nc.vector.tensor_scalar_mul(out=scc(dst[2]),
                                            in0=thc(srci + 2), scalar1=cmul)
                nc.vector.tensor_tensor(out=tmp1[:], in0=thc(srci + 3),
                                        in1=thc(srci), op=AO.subtract)
                nc.vector.tensor_tensor(out=tmp1[:], in0=tmp1[:],
                                        in1=thc(srci + 1), op=AO.subtract)
                nc.vector.tensor_tensor(out=tmp1[:], in0=tmp1[:],
                                        in1=thc(srci + 2), op=AO.subtract)
                nc.vector.tensor_scalar(out=scc(dst[3]), in0=tmp1[:],
                                        scalar1=osc, scalar2=osc + oadd,
                                        op0=AO.mult, op1=AO.add)

            mkrow((AX, BX, CX, OX), 0, r, 79.5, float(PADX))
            mkrow((AY, BY, CY, OY), 4, r, 79.5, float(PADY))
            mkrow((AZ, BZ, CZ, OZ), 8, 1.0, 47.5, float(PADZ))
            # z row: A=t20/r, B=t21/r, C=t22
            nc.vector.tensor_scalar_mul(out=scc(AZ), in0=thc(8),
                                        scalar1=1.0 / r)
            nc.vector.tensor_scalar_mul(out=scc(BZ), in0=thc(9),
                                        scalar1=1.0 / r)
            nc.vector.tensor_scalar_add(out=scc(AXM1), in0=scc(AX),
                                        scalar1=-1.0)

            def mkminb(dst, slope_i):
                nc.vector.tensor_scalar_mul(out=scc(dst), in0=scc(slope_i),
                                            scalar1=float(LX - 1))
                nc.vector.tensor_scalar_min(out=scc(dst), in0=scc(dst),
                                            scalar1=0.0)

            mkminb(MZB, AZ)
            mkminb(MYB, AY)
            mkminb(MXB, AXM1)

            # lane mapping: flat x-chunk index g = 40*p + t;
            # y = g // 32, xc = g % 32, x0 = 5*xc
            def iotaf(shape_pairs, n, tag, base=0, cm=0):
                ti_ = pre.tile([P, n], I32, tag=tag + "i")
                nc.gpsimd.iota(ti_[:], shape_pairs, base=base,
                               channel_multiplier=cm)
                tf_ = per.tile([P, n], F32, tag=tag + "f")
                nc.vector.tensor_copy(out=tf_[:], in_=ti_[:])
                return tf_

            gT = iotaf([[1, NT]], NT, "g", cm=NT)
            yT = per.tile([P, NT], F32)
            gi1 = pre.tile([P, NT], I32, tag="gi1")
            yv = pre.tile([P, NT], F32, tag="yv")
            nc.vector.tensor_scalar_mul(out=yv[:], in0=gT[:],
                                        scalar1=1.0 / 32.0)
            nc.vector.tensor_copy(out=gi1[:], in_=yv[:])
            nc.vector.tensor_copy(out=yT[:], in_=gi1[:])
            ygt = pre.tile([P, NT], F32, tag="ygt")
            nc.vector.tensor_tensor(out=ygt[:], in0=yT[:], in1=yv[:],
                                    op=AO.is_gt)
            nc.vector.tensor_tensor(out=yT[:], in0=yT[:], in1=ygt[:],
                                    op=AO.subtract)
            x0T = per.tile([P, NT], F32)
            nc.vector.scalar_tensor_tensor(out=x0T[:], in0=yT[:],
                                           scalar=-32.0, in1=gT[:],
                                           op0=AO.mult, op1=AO.add)
            nc.vector.tensor_scalar_mul(out=x0T[:], in0=x0T[:],
                                        scalar1=float(LX))

            # base40[axis] = A*x0 + B*y + O'   [P, NT]
            base40 = per.tile([P, 3, NT], F32)
            for ax, (ai, bi, oi) in enumerate(((AZ, BZ, OZ), (AY, BY, OY),
                                               (AX, BX, OX))):
                b1 = pre.tile([P, NT], F32, tag="b1")
                nc.vector.scalar_tensor_tensor(
                    out=b1[:], in0=x0T[:], scalar=scc(ai),
                    in1=fb(sc[:], [[0, NT]], oi), op0=AO.mult, op1=AO.add)
                nc.vector.scalar_tensor_tensor(
                    out=base40[:, ax, :], in0=yT[:], scalar=scc(bi),
                    in1=b1[:], op0=AO.mult, op1=AO.add)

            # weight templates T45[axis(z,y,x), K, wl] = slope*wl - K, fp16
            iwf = iotaf([[0, 3], [0, KX], [1, LX]], 45, "iw")
            ikf = iotaf([[0, 3], [1, KX], [0, LX]], 45, "ik")
            T45f = per.tile([P, 45], F32)
            for ax, si in enumerate((AZ, AY, AXM1)):
                nc.vector.scalar_tensor_tensor(
                    out=T45f[:, ax * 15:(ax + 1) * 15],
                    in0=iwf[:, ax * 15:(ax + 1) * 15], scalar=scc(si),
                    in1=ikf[:, ax * 15:(ax + 1) * 15],
                    op0=AO.mult, op1=AO.subtract)
            T45 = per.tile([P, 45], F16)
            nc.vector.tensor_copy(out=T45[:], in_=T45f[:])

            jzi = pre.tile([P, KZ], I32, tag="jzi")
            nc.gpsimd.iota(jzi[:], [[JZSTR, KZ]], base=0, channel_multiplier=0)
            jzTf = per.tile([P, KZ], F32)
            nc.vector.tensor_copy(out=jzTf[:], in_=jzi[:])

            # ---------- P3: per-task residuals (fp16) + PV row index ----
            rT = per.tile([P, NTL, 3], F16)
            idxT = per.tile([P, NTL], F32)
            lims = (ZLIM, YLIM, XLIM)
            mbs = (MZB, MYB, MXB)
            cxs = (CZ, CY, CX)
            for ch in range(NCHUNK):
                zoi = pre.tile([P, TPC], I32, tag="zoi")
                nc.gpsimd.iota(zoi[:], [[1, CH_SL], [0, NT]],
                               base=ch * CH_SL, channel_multiplier=0)
                zof = pre.tile([P, TPC], F32, tag="zof")
                nc.vector.tensor_copy(out=zof[:], in_=zoi[:])
                acc = idxT[:, ch * TPC:(ch + 1) * TPC]
                for ax in range(3):
                    cs = pre.tile([P, TPC], F32, tag="cs")
                    nc.vector.scalar_tensor_tensor(
                        out=cs[:], in0=zof[:], scalar=scc(cxs[ax]),
                        in1=fb(base40[:, ax, :], [[0, CH_SL], [1, NT]]),
                        op0=AO.mult, op1=AO.add)
                    bf = pre.tile([P, TPC], F32, tag="bf")
                    bv = pre.tile([P, TPC], F32, tag="bv")
                    bi_ = pre.tile([P, TPC], I32, tag="bi")
                    nc.vector.tensor_scalar_add(out=bv[:], in0=cs[:],
                                                scalar1=scc(mbs[ax]))
                    nc.vector.tensor_copy(out=bi_[:], in_=bv[:])
                    nc.vector.tensor_copy(out=bf[:], in_=bi_[:])
                    bg = pre.tile([P, TPC], F32, tag="bg")
                    nc.vector.tensor_tensor(out=bg[:], in0=bf[:], in1=bv[:],
                                            op=AO.is_gt)
                    nc.vector.tensor_tensor(out=bf[:], in0=bf[:], in1=bg[:],
                                            op=AO.subtract)
                    nc.vector.tensor_scalar_max(out=bf[:], in0=bf[:],
                                                scalar1=0.0)
                    nc.vector.tensor_scalar_min(out=bf[:], in0=bf[:],
                                                scalar1=lims[ax])
                    nc.vector.tensor_tensor(
                        out=fb(rT[:], [[3, TPC]], (ch * TPC) * 3 + ax),
                        in0=cs[:], in1=bf[:], op=AO.subtract)
                    if ax == 0:
                        nc.vector.tensor_scalar_mul(out=acc, in0=bf[:],
                                                    scalar1=float(JZSTR))
                    elif ax == 1:
                        nc.vector.scalar_tensor_tensor(
                            out=acc, in0=bf[:], scalar=float(XN), in1=acc,
                            op0=AO.mult, op1=AO.add)
                    else:
                        nc.vector.tensor_tensor(out=acc, in0=acc, in1=bf[:],
                                                op=AO.add)

            if DEBUG:
                nc.sync.dma_start(
                    out=dbg_pv[:],
                    in_=AP(pv[:].tensor, (50 * YPN + 80) * XN * INNER,
                           [[1, 128 * 4096]]))
                nc.sync.dma_start(out=AP(dbg_misc[:].tensor, 0,
                                         [[2 * NT + 3 * NT + 24, 128], [1, NT]]),
                                  in_=x0T[:])
                nc.sync.dma_start(out=AP(dbg_misc[:].tensor, NT,
                                         [[2 * NT + 3 * NT + 24, 128], [1, NT]]),
                                  in_=yT[:])
                nc.sync.dma_start(out=AP(dbg_misc[:].tensor, 2 * NT,
                                         [[2 * NT + 3 * NT + 24, 128], [1, 3 * NT]]),
                                  in_=base40[:].rearrange("p a b -> p (a b)"))
                nc.sync.dma_start(out=AP(dbg_misc[:].tensor, 5 * NT,
                                         [[2 * NT + 3 * NT + 24, 128], [1, 16]]),
                                  in_=sc[:, :16])
                nc.sync.dma_start(out=dbg_idx[:], in_=idxT[:])
                nc.sync.dma_start(out=dbg_r[:],
                                  in_=rT[:].rearrange("p a b -> p (a b)"))

            # ---------- P1: build PV ----------
            # partition = z (96 rows); jy shifts live in the free dim.
            # il[z, y0p-in-band, x, c, jy] = src[c, z, y0p-4+jy, x]
            YB = 8
            bands = [(b * YB, YB) for b in range(20)] + [(160, 4)]
            for (B, nb) in bands:
                scs = []
                for c in range(C):
                    sct = pre.tile([D, (YB + 2) * W], F32, tag=f"sc{c}")
                    r0 = B - PADY                  # first src row = B-4+jy=0
                    rlo, rhi = max(0, r0), min(H, r0 + YB + 2)
                    if rlo > r0 or rhi < r0 + YB + 2:
                        nc.vector.memset(sct[:], 0.0)
                    nc.sync.dma_start(
                        out=sct[:, (rlo - r0) * W:(rhi - r0) * W],
                        in_=AP(src[:].tensor, c * SRCEL + rlo * W,
                               [[HW, D], [W, rhi - rlo], [1, W]]))
                    scs.append(sct)
                il = pre.tile([D, YB * W * INNER], F16, tag="il")
                for c in range(C):
                    for jy in range(KY):
                        nc.scalar.activation(
                            fb(il[:], [[W * INNER, nb], [INNER, W]],
                               c * KY + jy),
                            fb(scs[c][:], [[W, nb], [1, W]], jy * W),
                            AF.Copy)
                dstoff = (PADZ * YPN + B) * XN * INNER + PADX * INNER
                nc.sync.dma_start(
                    out=AP(pv[:].tensor, dstoff,
                           [[YPN * XN * INNER, D], [XN * INNER, nb],
                            [1, W * INNER]]),
                    in_=fb(il[:], [[1, nb * W * INNER]]))

            # ---------- P4: main loop over z-slabs ----------
            pre_ctx.__exit__(None, None, None)
            dctx = tc.tile_pool(name="dp", bufs=2)
            dp = dctx.__enter__()
            actx = tc.tile_pool(name="ap", bufs=2)
            apl = actx.__enter__()
            wctx = tc.tile_pool(name="wp", bufs=1)
            wp = wctx.__enter__()
            octx = tc.tile_pool(name="op", bufs=2)
            op = octx.__enter__()

            stA = {}

            def stageA(sl):
                offf = dp.tile([P, NT, KZ], F32, tag="offf")
                nc.vector.tensor_tensor(
                    out=offf[:],
                    in0=fb(idxT[:], [[1, NT], [0, KZ]], sl * NT),
                    in1=fb(jzTf[:], [[0, NT], [1, KZ]]), op=AO.add)
                offs = dp.tile([P, NT * KZ], I32, tag="offs")
                nc.vector.tensor_copy(out=offs[:], in_=offf[:].rearrange(
                    "p a b -> p (a b)"))
                Dt = dp.tile([P, NT * KZ * DSTREAM], F16, tag="Dt")
                # HW indirect DMA honors exactly one offset per partition
                # per instruction -> one instruction per (task, jz) column
                for j in range(NT * KZ):
                    nc.gpsimd.indirect_dma_start(
                        out=Dt[:, j * DSTREAM:(j + 1) * DSTREAM],
                        out_offset=None, in_=pv[:],
                        in_offset=bass.IndirectOffsetOnAxis(
                            ap=offs[:, j:j + 1], axis=0))
                args = apl.tile([P, NT, 3, 15], F16, tag="args")
                nc.vector.tensor_tensor(
                    out=args[:],
                    in0=fb(T45[:], [[0, NT], [15, 3], [1, 15]]),
                    in1=fb(rT[:], [[3, NT], [1, 3], [0, 15]], sl * NT * 3),
                    op=AO.add)
                habs = apl.tile([P, NT * 45], F16, tag="habs")
                nc.scalar.activation(habs[:],
                                     args[:].rearrange("p a b c -> p (a b c)"),
                                     AF.Abs)
                hatt = apl.tile([P, NT * 45], F16, tag="hatt")
                nc.scalar.activation(hatt[:], habs[:], AF.Relu,
                                     bias=1.0, scale=-1.0)
                hxb = apl.tile([P, NT, LX, KX, 9], F16, tag="hxb")
                for s in range(KX):
                    nc.scalar.activation(
                        fb(hxb[:], [[135, NT], [27, LX], [1, 9]], s * 9),
                        fb(hatt[:], [[45, NT], [1, LX], [0, 9]], 30 + s * 5),
                        AF.Copy)
                if DEBUG and sl == 0:
                    nc.sync.dma_start(out=dbg_dt[:], in_=Dt[:])
                    nc.sync.dma_start(out=dbg_hat[:], in_=hatt[:])
                return offf, offs, Dt, hatt, hxb

            def stageB(sl, Dt, hatt, hxb):
                # ISA limit: <=3 free dims per operand -> split small dims out
                w2 = wp.tile([P, NT, LX, KZ, KY], F16, tag="w2")
                for jz in range(KZ):
                    nc.vector.tensor_tensor(
                        out=fb(w2[:], [[45, NT], [9, LX], [1, KY]], jz * KY),
                        in0=fb(hatt[:], [[45, NT], [1, LX], [0, KY]],
                               jz * LX),
                        in1=fb(hatt[:], [[45, NT], [1, LX], [5, KY]], 15),
                        op=AO.mult)
                w3 = wp.tile([P, NT, LX, KX, 9], F16, tag="w3")
                for s in range(KX):
                    nc.vector.tensor_tensor(
                        out=fb(w3[:], [[135, NT], [27, LX], [1, 9]], s * 9),
                        in0=fb(w2[:], [[45, NT], [9, LX], [1, 9]]),
                        in1=fb(hxb[:], [[135, NT], [27, LX], [1, 9]], s * 9),
                        op=AO.mult)
                prod = wp.tile([P, NT, C, LX, 27], F16, tag="prod")
                for s in range(KX):
                    for jz in range(KZ):
                        for c in range(C):
                            nc.vector.tensor_tensor(
                                out=fb(prod[:], [[C * LX * 27, NT], [27, LX],
                                                 [1, KY]],
                                       c * LX * 27 + s * 9 + jz * 3),
                                in0=fb(w3[:], [[LX * 27, NT], [27, LX],
                                               [1, KY]], s * 9 + jz * 3),
                                in1=fb(Dt[:], [[KZ * DSTREAM, NT], [INNER, LX],
                                               [1, KY]],
                                       jz * DSTREAM + s * INNER + c * KY),
                                op=AO.mult)
                red1 = wp.tile([P, NT * C * LX * 9], F16, tag="red1")
                with nc.allow_low_precision(reason="fp16 trilinear accum"):
                    nc.vector.tensor_reduce(
                        out=red1[:],
                        in_=fb(prod[:], [[KY, NT * C * LX * 9], [1, KY]]),
                        op=AO.add, axis=mybir.AxisListType.X)
                    # red2 stored [c, t, wl] so each c is one contiguous
                    # 200-el run per lane -> clean output descriptors
                    red2 = op.tile([P, C * NT * LX], F16, tag="red2")
                    nc.vector.tensor_reduce(
                        out=fb(red2[:], [[LX, NT], [NT * LX, C], [1, LX]]),
                        in_=fb(red1[:], [[C * LX * 9, NT], [LX * 9, C],
                                         [9, LX], [1, 9]]),
                        op=AO.add, axis=mybir.AxisListType.X)
                if DEBUG and sl == 0:
                    nc.sync.dma_start(
                        out=dbg_prod[:],
                        in_=prod[:].rearrange("p a b c d -> p (a b c d)"))
                    nc.sync.dma_start(out=dbg_red2[:], in_=red2[:])
                for c in range(C):
                    nc.gpsimd.dma_start(
                        out=AP(out[:].tensor, c * SRCEL + sl * HW,
                               [[NT * LX, P], [1, NT * LX]]),
                        in_=fb(red2[:], [[1, NT * LX]], c * NT * LX))

            for sl in range(SLABS):
                a = stageA(sl)
                if sl > 0:
                    pa = stA[sl - 1]
                    stageB(sl - 1, pa[2], pa[3], pa[4])
                stA[sl] = a
            pa = stA[SLABS - 1]
            stageB(SLABS - 1, pa[2], pa[3], pa[4])

            octx.__exit__(None, None, None)
            wctx.__exit__(None, None, None)
            actx.__exit__(None, None, None)
            dctx.__exit__(None, None, None)

    nc.compile()
    return nc


def kernel(src, theta):
    if "prog" not in _CACHE:
        _CACHE["prog"] = _build_program()
    nc = _CACHE["prog"]
    in_maps = []
    for i in range(N):
        in_maps.append({
            "src": np.ascontiguousarray(src[i].reshape(C, SRCEL),
                                        dtype=np.float32),
            "theta": np.ascontiguousarray(theta[i].reshape(1, 12),
                                          dtype=np.float32),
        })
    res = run_bass_kernel_spmd(nc, in_maps, core_ids=list(range(N)))
    o = np.empty((N, C, D, H, W), dtype=np.float32)
    for i in range(N):
        o[i] = res.results[i]["out"].reshape(C, D, H, W)
    return o
